# revision 42
# baseline (speedup 1.0000x reference)
"""Trainium2 Bass kernel for nn_CapsuleLayer (dynamic routing), v2.

Problem:  u_hat = einsum('bri,crio->cbro', x, W);  3 routing iterations
          (softmax over R, weighted sum, squash, agreement update).
Shapes:   x [256, 1152, 8] f32, W [10, 1152, 8, 16] f32 ->
          out [10, 256, 1, 1, 16] f32.

v2 design (8 NeuronCores, data-parallel over batch, B_loc = 32/core):
  * all matmul operands fp16 (PSUM accumulation fp32, logits L fp32);
  * it0 s-pass: 72 wide matmuls, stationary = x r-block [128,32],
    moving = W r-block over all 10 classes [128,160], out [b,(c,o)];
  * it1/2 s-pass: per class, stationary = W[rr,16], moving = y[rr,32];
  * agreement: classes spread over 4 PE row-strips (c = 4w+k -> strip k,
    wave w) so the four per-chunk g-matmuls run concurrently; fused
    multiply with b-replicated x and a fully contiguous i-reduction tree
    (wt/xrep stored i-outermost per 128-r chunk);
  * softmax normalization deferred: y = exp(L-max)*x unnormalized, the
    1/Z factor (Z via ones-stationary matmuls on transposed cw) is folded
    into squash; cw transposes per 128-r block on the PE;
  * per-wave interleaving: agreement wave w -> exp/transpose/y-build ->
    s-matmuls of wave w classes, so tensor work streams without stalls;
  * all weight tensors resident in SBUF (no streaming DMA inside loops),
    startup DMAs chunked so compute starts early.
"""

import sys
from contextlib import ExitStack

import numpy as np

sys.path.insert(0, "/opt/trn_rl_repo")

import concourse.bacc as bacc
import concourse.bass as bass
import concourse.mybir as mybir
import concourse.tile as tile
from concourse.bass_utils import run_bass_kernel_spmd

F32 = mybir.dt.float32
F16 = mybir.dt.float16
BF16 = mybir.dt.bfloat16
MUL = mybir.AluOpType.mult
ADD = mybir.AluOpType.add
AX = mybir.ActivationFunctionType

B, R, I, C, O = 256, 1152, 8, 10, 16
NC = 8
BL = B // NC          # 32 batch per core
Q = R // 128          # 9 r-blocks of 128 (also agreement chunks)
CO = C * O            # 160
RI = R * I            # 9216
EPS = 1e-7
W3 = 3                # waves: classes c = 4w+k, k<4 (w<2), k<2 (w=2)


def _nk(w):
    return 4 if w < 2 else 2


def build_nc(debug=False):
    nc = bacc.Bacc("TRN2", target_bir_lowering=False, debug=debug)

    xtr_d = nc.declare_dram_parameter("xtr", [128, Q, I, BL], F16, isOutput=False)
    wfr_d = nc.declare_dram_parameter("wfr", [128, Q, I, 5, 64], F16, isOutput=False)
    wta_d = nc.declare_dram_parameter("wta", [128, 2, Q, I, 128], F16, isOutput=False)
    wtb_d = nc.declare_dram_parameter("wtb", [64, Q, I, 128], F16, isOutput=False)
    xrep_d = nc.declare_dram_parameter("xrep", [32, I, Q, 128], F16, isOutput=False)
    ident_d = nc.declare_dram_parameter("ident", [128, 128], F32, isOutput=False)
    out_d = nc.declare_dram_parameter("out", [C, O, BL], F32, isOutput=True)

    with tile.TileContext(nc) as tc, ExitStack() as ctx:
        res = ctx.enter_context(tc.tile_pool(name="res", bufs=1))
        yp = ctx.enter_context(tc.tile_pool(name="yp", bufs=1))
        cwp = ctx.enter_context(tc.tile_pool(name="cwp", bufs=2))
        cvp = ctx.enter_context(tc.tile_pool(name="cvp", bufs=1))
        gmp = ctx.enter_context(tc.tile_pool(name="gmp", bufs=1))
        trp = ctx.enter_context(tc.tile_pool(name="trp", bufs=1))
        smp = ctx.enter_context(tc.tile_pool(name="smp", bufs=1))
        psG = ctx.enter_context(
            tc.tile_pool(name="psG", bufs=2, space=bass.MemorySpace.PSUM)
        )
        psT = ctx.enter_context(
            tc.tile_pool(name="psT", bufs=2, space=bass.MemorySpace.PSUM)
        )
        psS = ctx.enter_context(
            tc.tile_pool(name="psS", bufs=1, space=bass.MemorySpace.PSUM)
        )
        psZ = ctx.enter_context(
            tc.tile_pool(name="psZ", bufs=1, space=bass.MemorySpace.PSUM)
        )

        # ---- resident tensors -------------------------------------
        xtr = res.tile([128, Q, I, BL], F16)
        # paired-class W: cols (p, j): j 0:16 = W[2p], 32:48 = W[2p+1], rest 0
        wfr = res.tile([128, Q, I, 5, 64], F16)
        wta = res.tile([128, 2, Q, I, 128], F16)   # rows 32k+o: class 4w+k, w<2
        wtb = res.tile([64, Q, I, 128], F16)       # rows 32k+o: class 8+k
        # rows 32k+b: x[b, 128*n0+rl, i] stored i-major: [i, n0, rl]
        xrep = res.tile([128, I, Q, 128], F16)
        ident = res.tile([128, 128], F32)
        L = res.tile([128, W3, R], F32)            # logits, rows 32k+b

        vblk = res.tile([128, W3, 32], F16)        # rows 32k+o: v[4w+k, b, o]
        v_sb = res.tile([64, 5 * 64], F32)         # squash output, paired layout
        zir = res.tile([64, 5 * 64], F32)          # masked 1/Z, paired layout
        patE = res.tile([1, 48], F32)
        patO = res.tile([1, 16], F32)
        ones64f = res.tile([64, 1], F32)
        onescol = res.tile([1, 64], F32)
        ones128 = res.tile([128, 1], F16)

        # startup DMAs, ordered by first use (it0 needs xtr+wfr only)
        nc.sync.dma_start(xtr[:], xtr_d[:])
        for qg in range(3):
            nc.sync.dma_start(
                wfr[:, 3 * qg : 3 * qg + 3, :, :, :],
                wfr_d[:, 3 * qg : 3 * qg + 3, :, :, :],
            )
        nc.sync.dma_start(ident[:], ident_d[:])
        nc.sync.dma_start(wta[:, 0, :, :, :], wta_d[:, 0, :, :, :])
        nc.sync.dma_start(xrep[0:32, :, :, :], xrep_d[:])
        for kk in range(1, 4):
            nc.sync.dma_start(
                xrep[32 * kk : 32 * kk + 32, :, :, :], xrep[0:32, :, :, :]
            )
        nc.sync.dma_start(wta[:, 1, :, :, :], wta_d[:, 1, :, :, :])
        nc.sync.dma_start(wtb[:], wtb_d[:])
        nc.vector.memset(ones128[:], 1.0)
        nc.vector.memset(ones64f[:], 1.0)
        nc.vector.memset(onescol[:], 1.0)
        nc.vector.memset(L[:], 0.0)
        nc.vector.memset(zir[:], 0.0)
        nc.vector.memset(patE[:], 0.0)
        nc.vector.memset(patE[:, 0:16], 1.0)
        nc.vector.memset(patO[:], 1.0)

        sps = psS.tile([64, 5 * 64], F32, tag="sps")   # paired layout
        zps = psZ.tile([1, W3 * 128], F32, tag="zps")

        # ---------------------------------------------------------------
        def s_pass0():
            """sps0[b, (p,j)] = sum x*W in paired-W layout (gap cols -> 0)."""
            out = sps[0:32, :]
            for q in range(Q):
                for i in range(I):
                    nc.tensor.matmul(
                        out,
                        xtr[:, q, i, :],
                        wfr[:, q, i, :, :].rearrange("p a b -> p (a b)"),
                        start=(q == 0 and i == 0),
                        stop=(q == Q - 1 and i == I - 1),
                    )

        # =========================== flow ==============================

        # ---- iteration 0: uniform-weight s-pass + squash -------------
        s_pass0()

        # squash0 (layout [32 b, (p,h,o)] paired), folds the uniform 1/R
        sq0 = smp.tile([32, 5, 2, 32], F32, tag="sq64")
        nc.scalar.activation(
            sq0[:].rearrange("p a b c -> p (a b c)"), sps[0:32, :], AX.Square
        )
        sqv = sq0[:, :, :, 0:16]                      # valid o cols
        t1 = smp.tile([32, C, 8], F32, tag="t1")
        t1v = t1[:].rearrange("p (a b) j -> p a b j", b=2)
        t2 = smp.tile([32, C, 4], F32, tag="t2")
        sn0 = smp.tile([32, C, 2], F32, tag="sn0pre")
        snf = smp.tile([32, C], F32, tag="sn0")
        nc.vector.tensor_tensor(t1v[:], sqv[:, :, :, 0:8], sqv[:, :, :, 8:16], ADD)
        nc.vector.tensor_tensor(t2[:], t1[:, :, 0:4], t1[:, :, 4:8], ADD)
        nc.vector.tensor_tensor(sn0[:], t2[:, :, 0:2], t2[:, :, 2:4], ADD)
        nc.vector.tensor_tensor(
            snf[:].unsqueeze(2), sn0[:, :, 0:1], sn0[:, :, 1:2], ADD
        )
        nc.vector.tensor_scalar_mul(snf[:], snf[:], 1.0 / (R * R))
        u1 = smp.tile([32, C], F32, tag="u1s")
        u2 = smp.tile([32, C], F32, tag="u2s")
        u3 = smp.tile([32, C], F32, tag="u3s")
        f0 = smp.tile([32, C], F32, tag="f0s")
        nc.vector.tensor_scalar_add(u1[:], snf[:], EPS)
        nc.scalar.activation(u2[:], u1[:], AX.Sqrt)
        nc.vector.tensor_scalar_add(u3[:], snf[:], 1.0)
        nc.vector.tensor_tensor(u1[:], u2[:], u3[:], MUL)
        nc.vector.reciprocal(u2[:], u1[:])
        nc.vector.tensor_tensor(f0[:], snf[:], u2[:], MUL)
        nc.vector.tensor_scalar_mul(f0[:], f0[:], 1.0 / R)
        v0 = v_sb[0:32, 0:160].rearrange("p (c o) -> p c o", o=O)
        nc.vector.tensor_tensor(
            v0.rearrange("p (a b) o -> p a b o", b=2),
            sps[0:32, :].rearrange("p (a b o) -> p a b o", b=2, o=32)[:, :, :, 0:16],
            f0[:].unsqueeze(2).broadcast_to([32, C, O]).rearrange(
                "p (a b) o -> p a b o", b=2
            ),
            MUL,
        )
        # stage v0 -> vblk: pack per wave into [32 b, (k,o)] cols 32k+o,
        # then one PE transpose lands rows 32k+o = vblk layout directly.
        for w in range(W3):
            v0w = smp.tile([32, 4, 32], F32, tag="v0w")
            nc.vector.memset(v0w[:], 0.0)
            for k in range(_nk(w)):
                nc.vector.tensor_copy(v0w[:, k, 0:16], v0[:, 4 * w + k, :])
            tv = psT.tile([128, 384], F32, tag="tps")
            nc.tensor.transpose(
                tv[:, 0:32],
                v0w[:].rearrange("p k o -> p (k o)"),
                ident[0:32, 0:32],
            )
            nc.scalar.copy(vblk[:, w, :], tv[:, 0:32])

        # ---- boundaries: agreement(it) + softmax + y + s-pass(it+1) ----
        def agree_wave(it, w):
            """g-matmuls per 128-r chunk; elementwise merged per half-wave
            (chunks 0-4 / 5-8) so the two gd tiles pseudo-double-buffer."""
            nk = _nk(w)
            npart = 32 * nk
            for half, (c0, c1) in enumerate(((0, 5), (5, 9))):
                nch = c1 - c0
                gd = gmp.tile([128, I, nch, 128], F16, tag=f"gd{half}")
                for s in range(nch):
                    n0 = c0 + s
                    gps = psG.tile([128, 1024], F32, tag="gps")
                    for k in range(nk):
                        if w < 2:
                            wsrc = wta[32 * k : 32 * k + 16, w, n0, :, :]
                        else:
                            wsrc = wtb[32 * k : 32 * k + 16, n0, :, :]
                        for sub in range(2):
                            nc.tensor.matmul(
                                gps[32 * k : 32 * k + 32, 512 * sub : 512 * sub + 512],
                                vblk[32 * k : 32 * k + 16, w, :],
                                wsrc.rearrange("p i r -> p (i r)")[
                                    :, 512 * sub : 512 * sub + 512
                                ],
                                start=True,
                                stop=True,
                                tile_position=(32 * k, 32 * k),
                            )
                    nc.scalar.copy(
                        gd[0:npart, :, s, :],
                        gps[0:npart, :].rearrange("p (i r) -> p i r", i=I),
                    )
                W2 = nch * 128
                nc.vector.tensor_tensor(
                    gd[0:npart, :, :, :],
                    gd[0:npart, :, :, :],
                    xrep[0:npart, :, c0:c1, :],
                    MUL,
                )
                gm = gd[:, :, :, :].rearrange("p i s r -> p (i s r)")
                l1 = trp.tile([128, 2 * 640], F16, tag="l1")
                nc.vector.tensor_tensor(
                    l1[0:npart, 0 : 2 * W2],
                    gm[0:npart, 0 : 2 * W2],
                    gm[0:npart, 2 * W2 : 4 * W2],
                    ADD,
                )
                nc.vector.tensor_tensor(
                    l1[0:npart, 0 : 2 * W2],
                    l1[0:npart, 0 : 2 * W2],
                    gm[0:npart, 4 * W2 : 6 * W2],
                    ADD,
                )
                nc.vector.tensor_tensor(
                    l1[0:npart, 0 : 2 * W2],
                    l1[0:npart, 0 : 2 * W2],
                    gm[0:npart, 6 * W2 : 8 * W2],
                    ADD,
                )
                rsl = slice(128 * c0, 128 * c1)
                if it == 0:
                    nc.vector.tensor_tensor(
                        L[0:npart, w, rsl],
                        l1[0:npart, 0:W2],
                        l1[0:npart, W2 : 2 * W2],
                        ADD,
                    )
                else:
                    nc.vector.tensor_tensor(
                        L[0:npart, w, rsl], L[0:npart, w, rsl], l1[0:npart, 0:W2], ADD
                    )
                    nc.vector.tensor_tensor(
                        L[0:npart, w, rsl],
                        L[0:npart, w, rsl],
                        l1[0:npart, W2 : 2 * W2],
                        ADD,
                    )

        def softmax_y_wave(w):
            """exp(L - rowmax) -> cwT (transposed, fp16), Z matmuls, y build."""
            nk = _nk(w)
            npart = 32 * nk
            m = smp.tile([128, 1], F32, tag="rmax")
            nc.vector.reduce_max(m[0:npart, :], L[0:npart, w, :], axis=mybir.AxisListType.X)
            negm = smp.tile([128, 1], F32, tag="negm")
            nc.vector.tensor_scalar_mul(negm[0:npart, :], m[0:npart, :], -1.0)
            cwv = cvp.tile([128, R], F32, tag="cwv")
            nc.scalar.activation(
                cwv[0:npart, :], L[0:npart, w, :], AX.Exp, bias=negm[0:npart, :]
            )
            cwT = cwp.tile([128, Q, 128], F16, tag="cwT")
            for q in range(Q):
                tps = psT.tile([128, 384], F32, tag="tps")
                nc.tensor.transpose(
                    tps[:, 0:npart],
                    cwv[0:npart, 128 * q : 128 * q + 128],
                    ident[0:npart, 0:npart],
                )
                nc.scalar.copy(cwT[:, q, 0:npart], tps[:, 0:npart])
                nc.tensor.matmul(
                    zps[:, 128 * w : 128 * w + npart],
                    ones128[:],
                    cwT[:, q, 0:npart],
                    start=(q == 0),
                    stop=(q == Q - 1),
                )
            y = yp.tile([128, 4, Q, I, BL], BF16, tag="y")
            for k in range(nk):
                eng = nc.vector if k < 3 else nc.gpsimd
                eng.tensor_tensor(
                    y[:, k, :, :, :],
                    xtr[:, :, :, :],
                    cwT[:, :, 32 * k : 32 * k + 32]
                    .unsqueeze(2)
                    .broadcast_to([128, Q, I, BL]),
                    MUL,
                )
            return y

        def s_mm_wave(w, y):
            """paired s-matmuls: one MM per (pair, q, i), N=64."""
            for kp in range(_nk(w) // 2):
                p = 2 * w + kp
                k2 = 2 * kp
                for q in range(Q):
                    for i in range(I):
                        nc.tensor.matmul(
                            sps[:, 64 * p : 64 * p + 64],
                            wfr[:, q, i, p, :],
                            y[:, k2 : k2 + 2, q, i, :],
                            start=(q == 0 and i == 0),
                            stop=(q == Q - 1 and i == I - 1),
                        )

        def squash_it(last):
            """squash in the paired [64, 320] layout; masked 1/Z kills the
            off-diagonal garbage blocks; Z folded into f."""
            # masked zir: rows 0:16 = zi*maskE, rows 32:48 = zi*maskO
            zsb = smp.tile([1, C * BL], F32, tag="zsb")
            nc.scalar.copy(zsb[:], zps[:, 0 : C * BL])
            zi = smp.tile([1, C * BL], F32, tag="zi")
            zmE = smp.tile([1, C * BL], F32, tag="zmE")
            zmO = smp.tile([1, C * BL], F32, tag="zmO")
            nc.vector.reciprocal(zi[:], zsb[:])
            nc.vector.memset(zmE[:], 0.0)
            nc.vector.memset(zmO[:], 0.0)
            for p in range(5):
                nc.vector.tensor_copy(
                    zmE[:, 64 * p : 64 * p + 32], zi[:, 64 * p : 64 * p + 32]
                )
                nc.vector.tensor_copy(
                    zmO[:, 64 * p + 32 : 64 * p + 64], zi[:, 64 * p + 32 : 64 * p + 64]
                )
            tpz = psT.tile([128, 384], F32, tag="tps")
            nc.tensor.matmul(tpz[0:48, 0:320], patE[:], zmE[:], start=True, stop=True)
            nc.tensor.matmul(tpz[32:48, 0:320], patO[:], zmO[:], start=True, stop=True)
            nc.scalar.copy(zir[0:48, :], tpz[0:48, 0:320])
            s64 = v_sb  # compute s in the output buffer; v = s*f in place
            nc.vector.tensor_tensor(s64[:], sps[:], zir[:], MUL)
            sq64 = smp.tile([64, 5 * 64], F32, tag="sq64")
            nc.scalar.activation(sq64[:], s64[:], AX.Square)
            tps = psT.tile([128, 384], F32, tag="tps")
            nc.tensor.matmul(
                tps[0:1, 0:320], ones64f[:], sq64[:], start=True, stop=True
            )
            snu = smp.tile([1, C * BL], F32, tag="snu")
            nc.scalar.copy(snu[:], tps[0:1, 0:320])
            sn = snu
            a1 = zmE  # dead after pattern matmuls
            a2 = zmO
            a3 = zsb  # dead after zi
            f = smp.tile([1, C * BL], F32, tag="f")
            nc.vector.tensor_scalar_add(a1[:], sn[:], EPS)
            nc.scalar.activation(a2[:], a1[:], AX.Sqrt)
            nc.vector.tensor_scalar_add(a3[:], sn[:], 1.0)
            nc.vector.tensor_tensor(a1[:], a2[:], a3[:], MUL)
            nc.vector.reciprocal(a2[:], a1[:])
            nc.vector.tensor_tensor(f[:], sn[:], a2[:], MUL)
            tpf = psT.tile([128, 384], F32, tag="tps")
            nc.tensor.matmul(
                tpf[0:64, 0:320], onescol[:], f[:], start=True, stop=True
            )
            fr = sq64  # dead after the o-sum matmul
            nc.scalar.copy(fr[:], tpf[0:64, 0:320])
            nc.vector.tensor_tensor(v_sb[:], s64[:], fr[:], MUL)
            if not last:
                for c in range(C):
                    w, k = c // 4, c % 4
                    p, h = c // 2, c % 2
                    nc.scalar.copy(
                        vblk[32 * k : 32 * k + 16, w, :],
                        v_sb[32 * h : 32 * h + 16, 64 * p + 32 * h : 64 * p + 32 * h + 32],
                    )

        # boundaries: per-wave agree -> softmax/y -> s-MMs; the s-MM
        # stretch on tensor gives wave w's elementwise chain time to free
        # gd before wave w+1's drains need it
        for it in range(2):
            for w in range(W3):
                agree_wave(it, w)
            for w in range(W3):
                y = softmax_y_wave(w)
                s_mm_wave(w, y)
            squash_it(last=(it == 1))

        for h in range(2):
            nc.sync.dma_start(
                out_d[:].rearrange("(p h) o b -> h o p b", h=2)[h],
                v_sb[:].rearrange("(h s) (p g b) -> h s p g b", s=32, g=2, b=32)[
                    h, 0:16, :, h, :
                ],
            )

    nc.compile()
    return nc


# =================== host-side prep / entry point =====================

def _prep_shared(W):
    """Per-problem constant tensors (replicated on every core)."""
    W = np.ascontiguousarray(W, np.float32)
    # paired W: wfr[rr, q, i, p, 0:16] = W[2p], [, 32:48] = W[2p+1], rest 0
    wfl = W.reshape(C, Q, 128, I, O).transpose(2, 1, 3, 0, 4)  # [rr, q, i, c, o]
    wfr = np.zeros((128, Q, I, 5, 4, 16), np.float16)
    wfr[:, :, :, :, 0, :] = wfl[:, :, :, 0::2, :]
    wfr[:, :, :, :, 2, :] = wfl[:, :, :, 1::2, :]
    wfr = wfr.reshape(128, Q, I, 5, 64)
    # wta[32k+o, w, q, i, rl] = W[4w+k, 128q+rl, i, o]  (w<2)
    Wr = W.reshape(C, Q, 128, I, O)                      # [c, q, rl, i, o]
    wta = np.zeros((4, 32, 2, Q, I, 128), np.float16)
    for w in range(2):
        for k in range(4):
            wta[k, 0:16, w] = Wr[4 * w + k].transpose(3, 0, 2, 1)  # [o, q, i, rl]
    wta = wta.reshape(128, 2, Q, I, 128)
    # wtb[32k+o, q, i, rl] = W[8+k, 128q+rl, i, o]
    wtb = np.zeros((2, 32, Q, I, 128), np.float16)
    for k in range(2):
        wtb[k, 0:16] = Wr[8 + k].transpose(3, 0, 2, 1)
    wtb = wtb.reshape(64, Q, I, 128)
    ident = np.eye(128, dtype=np.float32)
    return wfr, wta, wtb, ident


def _prep_core(x_shard):
    """Per-core tensors for one 32-batch shard: xtr and xrep."""
    xs = np.ascontiguousarray(x_shard, np.float32)       # [32, 1152, 8]
    xq = xs.reshape(BL, Q, 128, I)                       # [b, q, rl, i]
    xtr = np.ascontiguousarray(xq.transpose(2, 1, 3, 0)).astype(np.float16)
    # xrep[32k+b, i, n0, rl] = x[b, 128*n0+rl, i]
    xg = xs.reshape(BL, Q, 128, I).transpose(0, 3, 1, 2)  # [b, i, n0, rl]
    xrep = np.ascontiguousarray(xg).astype(np.float16)  # [32, I, Q, 128]
    return xtr, xrep


def build_inmaps(x, W):
    wfr, wta, wtb, ident = _prep_shared(W)
    in_maps = []
    for m in range(NC):
        xtr, xrep = _prep_core(x[m * BL : (m + 1) * BL])
        in_maps.append(
            {"xtr": xtr, "wfr": wfr, "wta": wta, "wtb": wtb,
             "xrep": xrep, "ident": ident}
        )
    return in_maps


_NC_CACHE = {}


def kernel(x, W):
    x = np.asarray(x, np.float32)
    W = np.asarray(W, np.float32)
    if "nc" not in _NC_CACHE:
        _NC_CACHE["nc"] = build_nc()
    nc = _NC_CACHE["nc"]

    in_maps = build_inmaps(x, W)
    res = run_bass_kernel_spmd(nc, in_maps, list(range(NC)))
    out = np.empty((C, B, 1, 1, O), np.float32)
    for m in range(NC):
        o = res.results[m]["out"]                         # [C, O, BL]
        out[:, m * BL : (m + 1) * BL, 0, 0, :] = np.asarray(o).transpose(0, 2, 1)
    return out


if __name__ == "__main__":
    d = np.load("/root/problem/ref_data.npz")
    got = kernel(d["x"], d["W"])
    exp = d["expected"]
    err = np.abs(got - exp).max() / np.abs(exp).max()
    print("Relative error:", err)


# revision 43
# speedup vs baseline: 1.1906x; 1.1906x over previous
"""Trainium2 Bass kernel for nn_CapsuleLayer (dynamic routing), v2.

Problem:  u_hat = einsum('bri,crio->cbro', x, W);  3 routing iterations
          (softmax over R, weighted sum, squash, agreement update).
Shapes:   x [256, 1152, 8] f32, W [10, 1152, 8, 16] f32 ->
          out [10, 256, 1, 1, 16] f32.

v2 design (8 NeuronCores, data-parallel over batch, B_loc = 32/core):
  * all matmul operands fp16 (PSUM accumulation fp32, logits L fp32);
  * it0 s-pass: 72 wide matmuls, stationary = x r-block [128,32],
    moving = W r-block over all 10 classes [128,160], out [b,(c,o)];
  * it1/2 s-pass: per class, stationary = W[rr,16], moving = y[rr,32];
  * agreement: classes spread over 4 PE row-strips (c = 4w+k -> strip k,
    wave w) so the four per-chunk g-matmuls run concurrently; fused
    multiply with b-replicated x and a fully contiguous i-reduction tree
    (wt/xrep stored i-outermost per 128-r chunk);
  * softmax normalization deferred: y = exp(L-max)*x unnormalized, the
    1/Z factor (Z via ones-stationary matmuls on transposed cw) is folded
    into squash; cw transposes per 128-r block on the PE;
  * per-wave interleaving: agreement wave w -> exp/transpose/y-build ->
    s-matmuls of wave w classes, so tensor work streams without stalls;
  * all weight tensors resident in SBUF (no streaming DMA inside loops),
    startup DMAs chunked so compute starts early.
"""

import sys
from contextlib import ExitStack

import numpy as np

sys.path.insert(0, "/opt/trn_rl_repo")

import concourse.bacc as bacc
import concourse.bass as bass
import concourse.mybir as mybir
import concourse.tile as tile
from concourse.bass_utils import run_bass_kernel_spmd

F32 = mybir.dt.float32
F16 = mybir.dt.float16
BF16 = mybir.dt.bfloat16
MUL = mybir.AluOpType.mult
ADD = mybir.AluOpType.add
AX = mybir.ActivationFunctionType

B, R, I, C, O = 256, 1152, 8, 10, 16
NC = 8
BL = B // NC          # 32 batch per core
Q = R // 128          # 9 r-blocks of 128 (also agreement chunks)
CO = C * O            # 160
RI = R * I            # 9216
EPS = 1e-7
W3 = 3                # waves: classes c = 4w+k, k<4 (w<2), k<2 (w=2)


def _nk(w):
    return 4 if w < 2 else 2


def build_nc(debug=False):
    nc = bacc.Bacc("TRN2", target_bir_lowering=False, debug=debug)

    xtr_d = nc.declare_dram_parameter("xtr", [128, Q, I, BL], F16, isOutput=False)
    wfr_d = nc.declare_dram_parameter("wfr", [128, Q, I, CO], F16, isOutput=False)
    wta_d = nc.declare_dram_parameter("wta", [128, 2, Q, I, 128], F16, isOutput=False)
    wtb_d = nc.declare_dram_parameter("wtb", [64, Q, I, 128], F16, isOutput=False)
    xrep_d = nc.declare_dram_parameter("xrep", [32, I, Q, 128], F16, isOutput=False)
    ident_d = nc.declare_dram_parameter("ident", [128, 128], F32, isOutput=False)
    out_d = nc.declare_dram_parameter("out", [C, O, BL], F32, isOutput=True)

    with tile.TileContext(nc) as tc, ExitStack() as ctx:
        res = ctx.enter_context(tc.tile_pool(name="res", bufs=1))
        yp = ctx.enter_context(tc.tile_pool(name="yp", bufs=1))
        cwp = ctx.enter_context(tc.tile_pool(name="cwp", bufs=2))
        cvp = ctx.enter_context(tc.tile_pool(name="cvp", bufs=1))
        gmp = ctx.enter_context(tc.tile_pool(name="gmp", bufs=2))
        trp = ctx.enter_context(tc.tile_pool(name="trp", bufs=1))
        smp = ctx.enter_context(tc.tile_pool(name="smp", bufs=1))
        psG = ctx.enter_context(
            tc.tile_pool(name="psG", bufs=2, space=bass.MemorySpace.PSUM)
        )
        psT = ctx.enter_context(
            tc.tile_pool(name="psT", bufs=2, space=bass.MemorySpace.PSUM)
        )
        psS = ctx.enter_context(
            tc.tile_pool(name="psS", bufs=1, space=bass.MemorySpace.PSUM)
        )
        psZ = ctx.enter_context(
            tc.tile_pool(name="psZ", bufs=1, space=bass.MemorySpace.PSUM)
        )

        # ---- resident tensors -------------------------------------
        xtr = res.tile([128, Q, I, BL], F16)
        wfr = res.tile([128, Q, I, CO], F16)
        wta = res.tile([128, 2, Q, I, 128], F16)   # rows 32k+o: class 4w+k, w<2
        wtb = res.tile([64, Q, I, 128], F16)       # rows 32k+o: class 8+k
        # rows 32k+b: x[b, 128*n0+rl, i] stored i-major: [i, n0, rl]
        xrep = res.tile([128, I, Q, 128], F16)
        ident = res.tile([128, 128], F32)
        L = res.tile([128, W3, R], F32)            # logits, rows 32k+b

        vblk = res.tile([128, W3, 32], F16)        # rows 32k+o: v[4w+k, b, o]
        v_sb = res.tile([16, C * BL], F32)         # squash output [o, (c,b)]
        ones16f = res.tile([16, 1], F32)
        onesrow = res.tile([1, 16], F32)
        ones128 = res.tile([128, 1], F16)

        # startup DMAs, ordered by first use (it0 needs xtr+wfr only)
        nc.sync.dma_start(xtr[:], xtr_d[:])
        nc.sync.dma_start(wfr[:], wfr_d[:])
        nc.sync.dma_start(ident[:], ident_d[:])
        nc.sync.dma_start(wta[:, 0, :, :, :], wta_d[:, 0, :, :, :])
        nc.sync.dma_start(xrep[0:32, :, :, :], xrep_d[:])
        for kk in range(1, 4):
            nc.sync.dma_start(
                xrep[32 * kk : 32 * kk + 32, :, :, :], xrep[0:32, :, :, :]
            )
        nc.sync.dma_start(wta[:, 1, :, :, :], wta_d[:, 1, :, :, :])
        nc.sync.dma_start(wtb[:], wtb_d[:])
        nc.vector.memset(ones128[:], 1.0)
        nc.vector.memset(ones16f[:], 1.0)
        nc.vector.memset(onesrow[:], 1.0)
        nc.vector.memset(L[:], 0.0)

        sps = psS.tile([32, C * BL], F32, tag="sps")   # it0 [32,160] / it [16,320]
        zps = psZ.tile([1, W3 * 128], F32, tag="zps")

        # ---------------------------------------------------------------
        def s_pass0():
            """sps[b, (c,o)] = sum_{q,rr,i} x[b,128q+rr,i] * W[c,128q+rr,i,o]."""
            out = sps[:, 0:CO]
            for q in range(Q):
                for i in range(I):
                    nc.tensor.matmul(
                        out,
                        xtr[:, q, i, :],
                        wfr[:, q, i, :],
                        start=(q == 0 and i == 0),
                        stop=(q == Q - 1 and i == I - 1),
                    )

        # =========================== flow ==============================

        # ---- iteration 0: uniform-weight s-pass + squash -------------
        s_pass0()

        # squash0 (layout [32 b, (c,o)]), folds the uniform 1/R weight
        sq0 = smp.tile([32, C, O], F32, tag="sq0")
        nc.scalar.activation(
            sq0[:].rearrange("p c o -> p (c o)"), sps[:, 0:CO], AX.Square
        )
        t1 = smp.tile([32, C, 8], F32, tag="t1")
        t2 = smp.tile([32, C, 4], F32, tag="t2")
        sn0 = smp.tile([32, C, 2], F32, tag="sn0pre")
        snf = smp.tile([32, C], F32, tag="sn0")
        nc.vector.tensor_tensor(t1[:], sq0[:, :, 0:8], sq0[:, :, 8:16], ADD)
        nc.vector.tensor_tensor(t2[:], t1[:, :, 0:4], t1[:, :, 4:8], ADD)
        nc.vector.tensor_tensor(sn0[:], t2[:, :, 0:2], t2[:, :, 2:4], ADD)
        nc.vector.tensor_tensor(
            snf[:].unsqueeze(2), sn0[:, :, 0:1], sn0[:, :, 1:2], ADD
        )
        nc.vector.tensor_scalar_mul(snf[:], snf[:], 1.0 / (R * R))
        u1 = smp.tile([32, C], F32, tag="u1s")
        u2 = smp.tile([32, C], F32, tag="u2s")
        u3 = smp.tile([32, C], F32, tag="u3s")
        f0 = smp.tile([32, C], F32, tag="f0s")
        nc.vector.tensor_scalar_add(u1[:], snf[:], EPS)
        nc.scalar.activation(u2[:], u1[:], AX.Sqrt)
        nc.vector.tensor_scalar_add(u3[:], snf[:], 1.0)
        nc.vector.tensor_tensor(u1[:], u2[:], u3[:], MUL)
        nc.vector.reciprocal(u2[:], u1[:])
        nc.vector.tensor_tensor(f0[:], snf[:], u2[:], MUL)
        nc.vector.tensor_scalar_mul(f0[:], f0[:], 1.0 / R)
        v0 = smp.tile([32, C, O], F32, tag="v0")
        nc.vector.tensor_tensor(
            v0[:],
            sps[:, 0:CO].rearrange("p (c o) -> p c o", o=O),
            f0[:].unsqueeze(2).broadcast_to([32, C, O]),
            MUL,
        )
        # stage v0 -> vblk: pack per wave into [32 b, (k,o)] cols 32k+o,
        # then one PE transpose lands rows 32k+o = vblk layout directly.
        for w in range(W3):
            v0w = smp.tile([32, 4, 32], F32, tag="v0w")
            nc.vector.memset(v0w[:], 0.0)
            for k in range(_nk(w)):
                nc.vector.tensor_copy(v0w[:, k, 0:16], v0[:, 4 * w + k, :])
            tv = psT.tile([128, 384], F32, tag="tps")
            nc.tensor.transpose(
                tv[:, 0:32],
                v0w[:].rearrange("p k o -> p (k o)"),
                ident[0:32, 0:32],
            )
            nc.scalar.copy(vblk[:, w, :], tv[:, 0:32])

        # ---- boundaries: agreement(it) + softmax + y + s-pass(it+1) ----
        def agree_wave(it, w):
            """g-matmuls per 128-r chunk; elementwise merged per half-wave
            (chunks 0-4 / 5-8) so the two gd tiles pseudo-double-buffer."""
            nk = _nk(w)
            npart = 32 * nk
            for half, (c0, c1) in enumerate(((0, 5), (5, 9))):
                nch = c1 - c0
                gd = gmp.tile([128, I, nch, 128], F16, tag=f"gd{half}")
                for s in range(nch):
                    n0 = c0 + s
                    gps = psG.tile([128, 1024], F32, tag="gps")
                    for k in range(nk):
                        if w < 2:
                            wsrc = wta[32 * k : 32 * k + 16, w, n0, :, :]
                        else:
                            wsrc = wtb[32 * k : 32 * k + 16, n0, :, :]
                        for sub in range(2):
                            nc.tensor.matmul(
                                gps[32 * k : 32 * k + 32, 512 * sub : 512 * sub + 512],
                                vblk[32 * k : 32 * k + 16, w, :],
                                wsrc.rearrange("p i r -> p (i r)")[
                                    :, 512 * sub : 512 * sub + 512
                                ],
                                start=True,
                                stop=True,
                                tile_position=(32 * k, 32 * k),
                            )
                    nc.scalar.copy(
                        gd[0:npart, :, s, :],
                        gps[0:npart, :].rearrange("p (i r) -> p i r", i=I),
                    )
                W2 = nch * 128
                nc.vector.tensor_tensor(
                    gd[0:npart, :, :, :],
                    gd[0:npart, :, :, :],
                    xrep[0:npart, :, c0:c1, :],
                    MUL,
                )
                gm = gd[:, :, :, :].rearrange("p i s r -> p (i s r)")
                l1 = trp.tile([128, 2 * 640], F16, tag="l1")
                nc.vector.tensor_tensor(
                    l1[0:npart, 0 : 2 * W2],
                    gm[0:npart, 0 : 2 * W2],
                    gm[0:npart, 2 * W2 : 4 * W2],
                    ADD,
                )
                nc.vector.tensor_tensor(
                    l1[0:npart, 0 : 2 * W2],
                    l1[0:npart, 0 : 2 * W2],
                    gm[0:npart, 4 * W2 : 6 * W2],
                    ADD,
                )
                nc.vector.tensor_tensor(
                    l1[0:npart, 0 : 2 * W2],
                    l1[0:npart, 0 : 2 * W2],
                    gm[0:npart, 6 * W2 : 8 * W2],
                    ADD,
                )
                rsl = slice(128 * c0, 128 * c1)
                if it == 0:
                    nc.vector.tensor_tensor(
                        L[0:npart, w, rsl],
                        l1[0:npart, 0:W2],
                        l1[0:npart, W2 : 2 * W2],
                        ADD,
                    )
                else:
                    nc.vector.tensor_tensor(
                        L[0:npart, w, rsl], L[0:npart, w, rsl], l1[0:npart, 0:W2], ADD
                    )
                    nc.vector.tensor_tensor(
                        L[0:npart, w, rsl],
                        L[0:npart, w, rsl],
                        l1[0:npart, W2 : 2 * W2],
                        ADD,
                    )

        def softmax_y_wave(w):
            """exp(L - rowmax) -> cwT (transposed, fp16), Z matmuls, y build."""
            nk = _nk(w)
            npart = 32 * nk
            m = smp.tile([128, 1], F32, tag="rmax")
            nc.vector.reduce_max(m[0:npart, :], L[0:npart, w, :], axis=mybir.AxisListType.X)
            negm = smp.tile([128, 1], F32, tag="negm")
            nc.vector.tensor_scalar_mul(negm[0:npart, :], m[0:npart, :], -1.0)
            cwv = cvp.tile([128, R], F32, tag="cwv")
            nc.scalar.activation(
                cwv[0:npart, :], L[0:npart, w, :], AX.Exp, bias=negm[0:npart, :]
            )
            cwT = cwp.tile([128, Q, 128], F16, tag="cwT")
            for q in range(Q):
                tps = psT.tile([128, 384], F32, tag="tps")
                nc.tensor.transpose(
                    tps[:, 0:npart],
                    cwv[0:npart, 128 * q : 128 * q + 128],
                    ident[0:npart, 0:npart],
                )
                nc.scalar.copy(cwT[:, q, 0:npart], tps[:, 0:npart])
                nc.tensor.matmul(
                    zps[:, 128 * w : 128 * w + npart],
                    ones128[:],
                    cwT[:, q, 0:npart],
                    start=(q == 0),
                    stop=(q == Q - 1),
                )
            y = yp.tile([128, 4, Q, I, BL], BF16, tag="y")
            for k in range(nk):
                eng = nc.vector if k < 3 else nc.gpsimd
                eng.tensor_tensor(
                    y[:, k, :, :, :],
                    xtr[:, :, :, :],
                    cwT[:, :, 32 * k : 32 * k + 32]
                    .unsqueeze(2)
                    .broadcast_to([128, Q, I, BL]),
                    MUL,
                )
            return y

        def s_mm_wave(w, y):
            nk = _nk(w)
            spsv = sps[0:16, :].rearrange("p (c b) -> p c b", b=BL)
            for k in range(nk):
                c = 4 * w + k
                for q in range(Q):
                    for i in range(I):
                        nc.tensor.matmul(
                            spsv[:, c, :],
                            wfr[:, q, i, c * O : (c + 1) * O],
                            y[:, k, q, i, :],
                            start=(q == 0 and i == 0),
                            stop=(q == Q - 1 and i == I - 1),
                        )

        def squash_it(last):
            """squash with deferred softmax normalization (Z folded in)."""
            sq32 = smp.tile([16, C * BL], F32, tag="sq32")
            nc.scalar.activation(sq32[:], sps[0:16, :], AX.Square)
            tps = psT.tile([128, 384], F32, tag="tps")
            nc.tensor.matmul(
                tps[0:1, 0 : C * BL], ones16f[:], sq32[:], start=True, stop=True
            )
            zsb = smp.tile([1, C * BL], F32, tag="zsb")
            snu = smp.tile([1, C * BL], F32, tag="snu")
            nc.scalar.copy(zsb[:], zps[:, 0 : C * BL])
            nc.scalar.copy(snu[:], tps[0:1, 0 : C * BL])
            zi = smp.tile([1, C * BL], F32, tag="zi")
            zi2 = smp.tile([1, C * BL], F32, tag="zi2")
            sn = smp.tile([1, C * BL], F32, tag="sn")
            a1 = smp.tile([1, C * BL], F32, tag="a1x")
            nc.vector.reciprocal(zi[:], zsb[:])
            nc.vector.tensor_tensor(zi2[:], zi[:], zi[:], MUL)
            nc.vector.tensor_tensor(sn[:], snu[:], zi2[:], MUL)
            nc.vector.tensor_scalar_add(a1[:], sn[:], EPS)
            a2 = zi2  # dead after sn
            nc.scalar.activation(a2[:], a1[:], AX.Sqrt)
            a3 = zsb  # dead after zi
            f = smp.tile([1, C * BL], F32, tag="f")
            nc.vector.tensor_scalar_add(a3[:], sn[:], 1.0)
            nc.vector.tensor_tensor(a1[:], a2[:], a3[:], MUL)
            nc.vector.reciprocal(a2[:], a1[:])
            nc.vector.tensor_tensor(f[:], sn[:], a2[:], MUL)
            nc.vector.tensor_tensor(f[:], f[:], zi[:], MUL)  # fold 1/Z
            tpf = psT.tile([128, 384], F32, tag="tps")
            nc.tensor.matmul(
                tpf[0:16, 0 : C * BL], onesrow[:], f[:], start=True, stop=True
            )
            fr = smp.tile([16, C * BL], F32, tag="fr")
            nc.scalar.copy(fr[:], tpf[0:16, 0 : C * BL])
            nc.vector.tensor_tensor(v_sb[:], sps[0:16, :], fr[:], MUL)
            if not last:
                vv = v_sb[:].rearrange("p (c b) -> p c b", b=BL)
                for c in range(C):
                    w, k = c // 4, c % 4
                    nc.scalar.copy(vblk[32 * k : 32 * k + 16, w, :], vv[:, c, :])

        # boundaries: per-wave agree -> softmax/y -> s-MMs; the s-MM
        # stretch on tensor gives wave w's elementwise chain time to free
        # gd before wave w+1's drains need it
        for it in range(2):
            for w in range(W3):
                agree_wave(it, w)
            for w in range(W3):
                y = softmax_y_wave(w)
                s_mm_wave(w, y)
            squash_it(last=(it == 1))

        nc.sync.dma_start(
            out_d[:].rearrange("c o b -> o c b"),
            v_sb[:].rearrange("p (c b) -> p c b", b=BL),
        )

    nc.compile()
    return nc


# =================== host-side prep / entry point =====================

def _prep_shared(W):
    """Per-problem constant tensors (replicated on every core)."""
    W = np.ascontiguousarray(W, np.float32)
    # wfr[rr, q, i, 16c+o] = W[c, 128q+rr, i, o]
    wfr = np.ascontiguousarray(
        W.reshape(C, Q, 128, I, O).transpose(2, 1, 3, 0, 4).reshape(128, Q, I, CO)
    ).astype(np.float16)
    # wta[32k+o, w, q, i, rl] = W[4w+k, 128q+rl, i, o]  (w<2)
    Wr = W.reshape(C, Q, 128, I, O)                      # [c, q, rl, i, o]
    wta = np.zeros((4, 32, 2, Q, I, 128), np.float16)
    for w in range(2):
        for k in range(4):
            wta[k, 0:16, w] = Wr[4 * w + k].transpose(3, 0, 2, 1)  # [o, q, i, rl]
    wta = wta.reshape(128, 2, Q, I, 128)
    # wtb[32k+o, q, i, rl] = W[8+k, 128q+rl, i, o]
    wtb = np.zeros((2, 32, Q, I, 128), np.float16)
    for k in range(2):
        wtb[k, 0:16] = Wr[8 + k].transpose(3, 0, 2, 1)
    wtb = wtb.reshape(64, Q, I, 128)
    ident = np.eye(128, dtype=np.float32)
    return wfr, wta, wtb, ident


def _prep_core(x_shard):
    """Per-core tensors for one 32-batch shard: xtr and xrep."""
    xs = np.ascontiguousarray(x_shard, np.float32)       # [32, 1152, 8]
    xq = xs.reshape(BL, Q, 128, I)                       # [b, q, rl, i]
    xtr = np.ascontiguousarray(xq.transpose(2, 1, 3, 0)).astype(np.float16)
    # xrep[32k+b, i, n0, rl] = x[b, 128*n0+rl, i]
    xg = xs.reshape(BL, Q, 128, I).transpose(0, 3, 1, 2)  # [b, i, n0, rl]
    xrep = np.ascontiguousarray(xg).astype(np.float16)  # [32, I, Q, 128]
    return xtr, xrep


def build_inmaps(x, W):
    wfr, wta, wtb, ident = _prep_shared(W)
    in_maps = []
    for m in range(NC):
        xtr, xrep = _prep_core(x[m * BL : (m + 1) * BL])
        in_maps.append(
            {"xtr": xtr, "wfr": wfr, "wta": wta, "wtb": wtb,
             "xrep": xrep, "ident": ident}
        )
    return in_maps


_NC_CACHE = {}


def kernel(x, W):
    x = np.asarray(x, np.float32)
    W = np.asarray(W, np.float32)
    if "nc" not in _NC_CACHE:
        _NC_CACHE["nc"] = build_nc()
    nc = _NC_CACHE["nc"]

    in_maps = build_inmaps(x, W)
    res = run_bass_kernel_spmd(nc, in_maps, list(range(NC)))
    out = np.empty((C, B, 1, 1, O), np.float32)
    for m in range(NC):
        o = res.results[m]["out"]                         # [C, O, BL]
        out[:, m * BL : (m + 1) * BL, 0, 0, :] = np.asarray(o).transpose(0, 2, 1)
    return out


if __name__ == "__main__":
    d = np.load("/root/problem/ref_data.npz")
    got = kernel(d["x"], d["W"])
    exp = d["expected"]
    err = np.abs(got - exp).max() / np.abs(exp).max()
    print("Relative error:", err)
